# revision 25
# baseline (speedup 1.0000x reference)
"""Tensor-parallel causal attention kernel for 8 trn2 NeuronCores.

Problem: B=2, S=2048, H=2048, 16 heads, head_dim=128 fp32.
  qkv = hidden @ w_qkv.T ; causal attention ; out = attn @ w_o.T

Sharding (hardcoded): core c in 0..7 handles batch b=c//4 and heads
hs = [4*(c%4) .. 4*(c%4)+3].  Each core computes a partial o_proj
output (contraction over its 512 hidden dims); the host sums the 4
partials per batch and transposes.  No device collectives.

v2 design:
  * Projections (qkv + o_proj) run as fp8-e4m3 DoubleRow matmuls on
    hi/lo decomposed operands: value = e4m3(hi) + e4m3(lo) carries
    ~11 significant bits.  Computing hi*hi (paired across two k-blocks
    per instruction) plus the two cross terms costs 0.75 cycles/row —
    1.33x faster than bf16/fp16 — at near-fp16 accuracy (the dropped
    lo*lo term is ~1e-4 relative).
  * Attention core (QK^T, exp, PV, row-sums) in fp16/bf16.
  * Causal mask post-exp on gpsimd; softmax denominators broadcast via
    a rank-1 f32r matmul; reciprocal via a single approximate DVE op;
    normalize + hi/lo split of attn for the o_proj feeds gpsimd/DVE.
  * o_proj emitted into a deferred queue drained between attention
    blocks so its matmuls fill tensor-queue gaps; fp16 output partials
    (host sums in f32).

Device layout (host-pretiled, partition-major; j indexes fp8 planes):
  xt  [128,16,2,2048] e4 : xt[p,ko,j,s] = (hi,lo)[j] of hidden[b,s,ko*128+p]
  wq  [128,16,2, 512] e4 : wq[p,ko,j,o] = (lo,hi)[j] of 256*w_qkv[q_rows[o],ko*128+p]
  wk,wv same as wq (k_rows / v_rows)
  wo  [128, 4,2,2048] e4 : wo[p,kb,j,o] = (lo,hi)[j] of 256*w_o[o,cols[kb*128+p]]
  outt[128,16,2048] f16 : outt[p,ot,s] = outT_partial[ot*128+p, s]

Toolchain quirks worked around here (walrus 1-sync-wait slots):
  - chunked tail drain monkeypatch; NoOp splitting of multi-waits
  - f32r matmul operands must be produced as f32r by ACT/DVE
  - lower_extended_insts() for custom-DVE ISA instruction bytes
"""
import numpy as np

import concourse.bass as bass
import concourse.mybir as mybir
import concourse.tile as tile
from concourse.bass_utils import run_bass_kernel_spmd
from concourse.vector_clock import ScopedClock, VectorClock

P = 128
S = 2048
H = 2048
NH_LOCAL = 4          # heads per core
KO = H // P           # 16 contraction chunks for the projections
SQ = 512              # q chunk width
NQC = S // SQ         # 4 q chunks
NKB = S // P          # 16 key blocks
F32 = mybir.dt.float32
F32R = mybir.dt.float32r
F16 = mybir.dt.float16
BF = mybir.dt.bfloat16
F8 = mybir.dt.float8e4
DR = mybir.MatmulPerfMode.DoubleRow
AF = mybir.ActivationFunctionType
SW = 256.0            # fp8 weight scale
SCALE = 1.0 / float(np.sqrt(128.0)) / (SW * SW)
INV_SW = 1.0 / SW

XCH = 512             # x chunk width in phase 1
NXCH = S // XCH       # 4 chunks
HI_X, LO_X = 0, 1     # x / attn plane order
LO_W, HI_W = 0, 1     # weight plane order


def _drain_and_barrier_chunked(self, tick_clock, wait_clock, _MAX=1):
    """Split the kernel-tail drain's waits: walrus allows only one sync
    wait per instruction in this toolchain."""
    g = tick_clock.global_clock
    n = len(g)
    vals = [g[i] for i in range(n)]
    nz = [i for i, v in enumerate(vals) if v > 0]
    chunks = [nz[i:i + _MAX] for i in range(0, len(nz), _MAX)] or [[]]
    for chunk in chunks:
        vec = [vals[i] if i in chunk else 0 for i in range(n)]
        d = self.nc.sync.drain()
        wait_clock.add_sem_waits(d.ins, ScopedClock({None: VectorClock(vec)}))
    self.nc.all_engine_barrier()
    assert self.sems is not None
    popped = self.nc._tile_sem_poison_stack.pop()
    assert popped is self._sem_poison
    self.nc.clear_and_free_semaphores(list(self.sems.allocated().values()))
    self.nc.all_engine_barrier()


tile.TileContext._drain_and_barrier = _drain_and_barrier_chunked


def _split_multi_waits(nc):
    """walrus allows ONE sync wait per instruction: hoist extra waits onto
    same-engine NoOps inserted directly before the offending instruction
    (identical semantics — the engine queue blocks on each in turn)."""
    ctr = 0
    for f in nc.m.functions:
        for blk in f.blocks:
            new = []
            changed = False
            for inst in blk.instructions:
                si = inst.sync_info
                waits = list(si.on_wait) if si and si.on_wait else []
                if len(waits) > 1:
                    changed = True
                    for w in waits[:-1]:
                        ctr += 1
                        nop = mybir.InstNoOp(name=f"I-wsplit-{ctr}",
                                             engine=inst.engine,
                                             ins=[], outs=[])
                        nop.sync_info = mybir.SyncInfo(on_wait=[w],
                                                       on_update=[])
                        new.append(nop)
                    ups = list(si.on_update) if si.on_update else []
                    inst.sync_info = mybir.SyncInfo(on_wait=[waits[-1]],
                                                   on_update=ups)
                new.append(inst)
            if changed:
                blk.instructions = new
    return ctr


def build():
    nc = bass.Bass()
    xt = nc.dram_tensor("xt", [P, KO, 2, S], F8, kind="ExternalInput")
    wq = nc.dram_tensor("wq", [P, KO, 2, NH_LOCAL * P], F8,
                        kind="ExternalInput")
    wk = nc.dram_tensor("wk", [P, KO, 2, NH_LOCAL * P], F8,
                        kind="ExternalInput")
    wv = nc.dram_tensor("wv", [P, KO, 2, NH_LOCAL * P], F8,
                        kind="ExternalInput")
    wo = nc.dram_tensor("wo", [P, NH_LOCAL, 2, S], F8, kind="ExternalInput")
    outt = nc.dram_tensor("outt", [P, KO, S], F16, kind="ExternalOutput")

    with tile.TileContext(nc) as tc:
        from contextlib import ExitStack
        with ExitStack() as ctx:
            const = ctx.enter_context(tc.tile_pool(name="const", bufs=1))

            # ---- constants -------------------------------------------------
            ones_f = const.tile([P, 1], F32)
            nc.vector.memset(ones_f[:], 1.0)
            ones_b = const.tile([P, 1], BF)
            nc.scalar.copy(ones_b[:], ones_f[:])
            onesrow_f = const.tile([1, P], F32)
            nc.vector.memset(onesrow_f[:], 1.0)
            onesrow_r = const.tile([1, P], F32R)
            nc.scalar.copy(onesrow_r[:], onesrow_f[:])
            obs_dve = const.tile([1, 1], F32)
            nc.vector.memset(obs_dve[:], 0.0)

            def _one(ap):
                return ap[tuple(slice(0, 1) for _ in ap.shape)]

            def dve_war_touch(ap):
                nc.vector.tensor_copy(_one(ap[:]), obs_dve[:])

            # ---- residents (live through phase 1+2) -----------------------
            qkv_pool = ctx.enter_context(tc.tile_pool(name="qkvp", bufs=1))
            # Q,K as qkvT: [d_in, o_tile(0-3 Q heads, 4-7 K heads), s],
            # fp16, scaled by SW (absorbed into the exp scale)
            qk_sb = qkv_pool.tile([P, 2 * NH_LOCAL, S], F16)
            # V as [s_in, s_tile, d_local], fp16, natural scale
            v_sb = qkv_pool.tile([P, NKB, NH_LOCAL * P], F16)

            # ================= phase 1: QKV projection =====================
            # fp8 DoubleRow with hi/lo planes: per k-block pair (k,k+1)
            # three DR instructions cover hi*hi(k)+hi*hi(k+1),
            # lo*hi(k)+hi*lo(k), lo*hi(k+1)+hi*lo(k+1).
            # w free layout: [0:512]=Q, [512:1024]=K, [1024:1536]=V
            with tc.tile_pool(name="p1w", bufs=1) as p1w, \
                 tc.tile_pool(name="p1x", bufs=3) as p1x, \
                 tc.tile_pool(name="p1ps", bufs=4, space="PSUM") as p1ps:

                w_r = p1w.tile([P, KO, 2, 3 * NH_LOCAL * P], F8, tag="wr")
                x_tiles = []
                x_r0 = p1x.tile([P, KO, 2, XCH], F8, tag="xr", name="xr0")
                for kq in range(4):
                    ks = slice(4 * kq, 4 * (kq + 1))
                    nc.sync.dma_start(x_r0[:, ks], xt.ap()[:, ks, :, 0:XCH])
                    nc.sync.dma_start(w_r[:, ks, :, 0:4 * P], wq.ap()[:, ks])
                x_tiles.append(x_r0)
                for i, wdram in ((1, wk), (2, wv)):
                    nc.sync.dma_start(
                        w_r[:, :, :, 4 * i * P:4 * (i + 1) * P], wdram.ap())

                def proj_group(ps, w_cols, x_r, x_cols, w_is_lhs):
                    """24 DR matmuls accumulating w.T @ x (or x.T @ w)."""
                    n = 0
                    for kp in range(KO // 2):
                        k = 2 * kp
                        terms = []
                        # hi*hi for blocks k and k+1
                        terms.append((
                            w_r[:, k:k + 2, HI_W, w_cols],
                            x_r[:, k:k + 2, HI_X, x_cols]))
                        # cross terms for block k and k+1:
                        # plane0 = w_lo * x_hi, plane1 = w_hi * x_lo
                        for kk in (k, k + 1):
                            terms.append((
                                w_r[:, kk, :, w_cols],
                                x_r[:, kk, :, x_cols]))
                        for wap, xap in terms:
                            lhsT, rhs = (wap, xap) if w_is_lhs else (xap, wap)
                            nc.tensor.matmul(
                                ps[:], lhsT, rhs,
                                start=(n == 0), stop=(n == 23),
                                perf_mode=DR)
                            n += 1

                for xc in range(NXCH):
                    if xc > 0:
                        x_r = p1x.tile([P, KO, 2, XCH], F8, tag="xr")
                        nc.sync.dma_start(
                            x_r[:], xt.ap()[:, :, :, xc * XCH:(xc + 1) * XCH])
                    else:
                        x_r = x_tiles[0]

                    for ot in range(2 * NH_LOCAL):  # Q then K o-tiles
                        ps = p1ps.tile([P, XCH], F32, tag="p1qk")
                        proj_group(ps, slice(ot * P, (ot + 1) * P),
                                   x_r, slice(0, XCH), w_is_lhs=True)
                        nc.vector.tensor_copy(
                            qk_sb[:, ot, xc * XCH:(xc + 1) * XCH], ps[:])
                    # V: out [s_tile(128), d(512)]; psum is SW*v — ACT
                    # copy rescales to natural-scale fp16
                    for st in range(XCH // P):
                        stg = xc * (XCH // P) + st
                        ps = p1ps.tile([P, NH_LOCAL * P], F32, tag="p1v")
                        proj_group(
                            ps, slice(2 * NH_LOCAL * P, 3 * NH_LOCAL * P),
                            x_r, slice(st * P, (st + 1) * P), w_is_lhs=False)
                        nc.scalar.mul(v_sb[:, stg, :], ps[:], INV_SW)

            # ============ phase 2+3: attention + interleaved o_proj ========
            attn_pool = ctx.enter_context(tc.tile_pool(name="attnp", bufs=1))
            # attn in fp8 hi/lo planes for the o_proj DR matmuls
            attnT = attn_pool.tile([P, NH_LOCAL, 2, S], F8)

            p3w = ctx.enter_context(tc.tile_pool(name="p3w", bufs=1))
            wo_r = p3w.tile([P, NH_LOCAL, 2, S], F8)
            nc.sync.dma_start(wo_r[:], wo.ap())

            p2sb = ctx.enter_context(tc.tile_pool(name="p2sb", bufs=2))
            p2est = ctx.enter_context(tc.tile_pool(name="p2est", bufs=6))
            p2st = ctx.enter_context(
                tc.tile_pool(name="p2st", bufs=2, space="PSUM"))
            p2at = ctx.enter_context(
                tc.tile_pool(name="p2at", bufs=2, space="PSUM"))
            p2sm = ctx.enter_context(
                tc.tile_pool(name="p2sm", bufs=2, space="PSUM"))
            p3ps = ctx.enter_context(
                tc.tile_pool(name="p3ps", bufs=2, space="PSUM"))
            p3sb = ctx.enter_context(tc.tile_pool(name="p3sb", bufs=4))

            # Deferred-work queue: normalization chains and o_proj groups
            # are spread across later kb slots so their tensor ops fill
            # pipeline gaps instead of blocking the in-order tensor queue.
            pending = []
            slot = [0]

            def drain_slot():
                slot[0] += 1
                if pending and (len(pending) > 8 or slot[0] % 2 == 0):
                    pending.pop(0)()

            def make_norm(at_ps, den_r, h, qs):
                def norm():
                    rep_ps = p3ps.tile([P, SQ], F32, tag="p3ps")
                    nc.tensor.matmul(rep_ps[:], onesrow_r[:], den_r[:],
                                     start=True, stop=True)
                    rep_sb = p2sb.tile([P, SQ], F32, tag="repsb")
                    nc.vector.reciprocal_approx_fast(rep_sb[:], rep_ps[:])
                    a32 = p2sb.tile([P, SQ], F32, tag="a32")
                    nc.vector.tensor_mul(a32[:], at_ps[:], rep_sb[:])
                    # hi/lo fp8 split (DVE)
                    hi = attnT[:, h, HI_X, qs:qs + SQ]
                    nc.vector.tensor_copy(hi, a32[:])
                    nc.vector.scalar_tensor_tensor(
                        attnT[:, h, LO_X, qs:qs + SQ], a32[:], 1.0, hi,
                        mybir.AluOpType.mult, mybir.AluOpType.subtract)
                return norm

            def make_oproj(sc, ot):
                def oproj():
                    ps = p3ps.tile([P, SQ], F32, tag="p3ps")
                    n = 0
                    for kp in (0, 2):   # hi*hi over kb pairs
                        nc.tensor.matmul(
                            ps[:],
                            wo_r[:, kp:kp + 2, HI_W, ot * P:(ot + 1) * P],
                            attnT[:, kp:kp + 2, HI_X, sc * SQ:(sc + 1) * SQ],
                            start=(n == 0), stop=False, perf_mode=DR)
                        n += 1
                    for kb in range(NH_LOCAL):  # cross terms
                        nc.tensor.matmul(
                            ps[:],
                            wo_r[:, kb, :, ot * P:(ot + 1) * P],
                            attnT[:, kb, :, sc * SQ:(sc + 1) * SQ],
                            start=False, stop=(kb == NH_LOCAL - 1),
                            perf_mode=DR)
                    stage = p3sb.tile([P, SQ], F16, tag="p3stage")
                    dve_war_touch(stage)
                    nc.vector.tensor_scalar_mul(stage[:], ps[:], INV_SW)
                    nc.sync.dma_start(
                        outt.ap()[:, ot, sc * SQ:(sc + 1) * SQ],
                        stage[:])
                return oproj

            def att_main(h, qc):
                nkb = 4 * (qc + 1)
                qs = qc * SQ

                at_ps = p2at.tile([P, SQ], F32, tag="atps")
                sm_ps = p2sm.tile([1, SQ], F32, tag="smps")

                st_tiles = {}

                def emit_st(kb):
                    st_ps = p2st.tile([P, SQ], F32, tag="stps")
                    nc.tensor.matmul(
                        st_ps[:],
                        qk_sb[:, NH_LOCAL + h, kb * P:(kb + 1) * P],
                        qk_sb[:, h, qs:qs + SQ],
                        start=True, stop=True)
                    st_tiles[kb] = st_ps

                emit_st(0)
                emit_st(1)
                for kb in range(nkb):
                    drain_slot()
                    if kb + 2 < nkb:
                        emit_st(kb + 2)
                    st_ps = st_tiles.pop(kb)
                    est = p2est.tile([P, SQ], BF, tag="est")
                    nc.scalar.activation(est[:], st_ps[:], AF.Exp,
                                         scale=SCALE)
                    if kb * P + P - 1 > qs:  # crosses the causal diagonal
                        nc.gpsimd.affine_select(
                            est[:], est[:], [[1, SQ]],
                            mybir.AluOpType.is_ge, 0.0,
                            base=qs - kb * P,
                            channel_multiplier=-1)
                    nc.tensor.matmul(sm_ps[:], ones_b[:], est[:],
                                     start=(kb == 0),
                                     stop=(kb == nkb - 1))
                    nc.tensor.matmul(
                        at_ps[:],
                        v_sb[:, kb, h * P:(h + 1) * P],
                        est[:],
                        start=(kb == 0), stop=(kb == nkb - 1))

                # denominators to f32r right away (ACT queue, lands just
                # after this head's last exp); the rest of the normalize
                # chain is deferred into the next head's kb slots.
                den_r = p2sb.tile([1, SQ], F32R, tag="denr")
                nc.scalar.copy(den_r[:], sm_ps[:])
                pending.append(make_norm(at_ps, den_r, h, qs))

            for qc in range(NQC):
                for h in range(NH_LOCAL):
                    att_main(h, qc)
                pending.extend(make_oproj(qc, ot) for ot in range(KO))
            while pending:
                pending.pop(0)()

    from concourse.library_overlay import lower_extended_insts
    lower_extended_insts(nc)   # populate .instr bytes for custom ISA ops
    _split_multi_waits(nc)
    return nc


_NC_CACHE = None


def _get_nc():
    global _NC_CACHE
    if _NC_CACHE is None:
        _NC_CACHE = build()
    return _NC_CACHE


def _hilo(a32, first):
    """Split fp32 array into two e4m3 planes stacked on a new axis -3.
    first='hi' gives (hi, lo); first='lo' gives (lo, hi)."""
    import ml_dtypes
    E4 = ml_dtypes.float8_e4m3fn
    hi = np.clip(a32, -240, 240).astype(E4)
    lo = np.clip(a32 - hi.astype(np.float32), -240, 240).astype(E4)
    pair = (hi, lo) if first == "hi" else (lo, hi)
    return np.stack(pair, axis=-2)


def _prep_inputs(hidden_states, w_qkv, w_o):
    """Host-side shard + pre-tile + hi/lo fp8 split for the 8 cores."""
    hidden_states = np.asarray(hidden_states, dtype=np.float32)
    w_qkv = np.asarray(w_qkv, dtype=np.float32) * SW
    w_o = np.asarray(w_o, dtype=np.float32) * SW
    B = hidden_states.shape[0]

    in_maps = []
    xt_by_b = {}
    for b in range(B):
        # [p, ko, s] = hidden[b, s, ko*128+p], then hi/lo planes
        xt32 = np.ascontiguousarray(
            hidden_states[b].T.reshape(KO, P, S).transpose(1, 0, 2))
        xt_by_b[b] = np.ascontiguousarray(_hilo(xt32, "hi"))
    for c in range(8):
        b = c // 4
        hs = [4 * (c % 4) + j for j in range(NH_LOCAL)]
        q_rows = np.concatenate([np.arange(h * P, (h + 1) * P) for h in hs])
        k_rows = q_rows + H
        v_rows = q_rows + 2 * H

        def wtile(rows):
            # [p, ko, o] = SW*w_qkv[rows[o], ko*128+p], then lo/hi planes
            w = w_qkv[rows, :]                      # [512, 2048]
            w32 = np.ascontiguousarray(
                w.T.reshape(KO, P, len(rows)).transpose(1, 0, 2))
            return np.ascontiguousarray(_hilo(w32, "lo"))

        # wo[p, kb, o] = SW*w_o[o, cols[kb*128+p]], then lo/hi planes
        wo32 = np.ascontiguousarray(
            w_o[:, q_rows].T.reshape(NH_LOCAL, P, S).transpose(1, 0, 2))
        wo_c = np.ascontiguousarray(_hilo(wo32, "lo"))
        in_maps.append({
            "xt": xt_by_b[b],
            "wq": wtile(q_rows),
            "wk": wtile(k_rows),
            "wv": wtile(v_rows),
            "wo": wo_c,
        })
    return in_maps


def run(hidden_states, w_qkv, w_o, trace=False, trace_cores=None):
    in_maps = _prep_inputs(hidden_states, w_qkv, w_o)
    nc = _get_nc()
    kwargs = {}
    if trace:
        kwargs["trace_cores"] = (trace_cores if trace_cores is not None
                                 else list(range(8)))
    res = run_bass_kernel_spmd(nc, in_maps, core_ids=list(range(8)),
                               trace=trace, **kwargs)
    B, S_, H_ = np.asarray(hidden_states).shape
    out = np.zeros((B, S_, H_), dtype=np.float32)
    for c in range(8):
        b = c // 4
        outt = res.results[c]["outt"]               # [128, 16, 2048] fp16
        outT = outt.astype(np.float32).transpose(1, 0, 2).reshape(H_, S_)
        out[b] += outT.T
    return out, res


def kernel(hidden_states, w_qkv, w_o):
    out, _ = run(hidden_states, w_qkv, w_o, trace=False)
    return out


# revision 26
# speedup vs baseline: 1.2616x; 1.2616x over previous
"""Tensor-parallel causal attention kernel for 8 trn2 NeuronCores.

Problem: B=2, S=2048, H=2048, 16 heads, head_dim=128 fp32.
  qkv = hidden @ w_qkv.T ; causal attention ; out = attn @ w_o.T

Sharding (hardcoded): core c in 0..7 handles batch b=c//4 and heads
hs = [4*(c%4) .. 4*(c%4)+3].  Each core computes a partial o_proj
output (contraction over its 512 hidden dims); the host sums the 4
partials per batch and transposes.  No device collectives.

v2 design:
  * Projections (qkv + o_proj) run as fp8-e4m3 DoubleRow matmuls on
    hi/lo decomposed operands: value = e4m3(hi) + e4m3(lo) carries
    ~11 significant bits.  Computing hi*hi (paired across two k-blocks
    per instruction) plus the two cross terms costs 0.75 cycles/row —
    1.33x faster than bf16/fp16 — at near-fp16 accuracy (the dropped
    lo*lo term is ~1e-4 relative).
  * Attention core (QK^T, exp, PV, row-sums) in fp16/bf16.
  * Causal mask post-exp on gpsimd; softmax denominators broadcast via
    a rank-1 f32r matmul; reciprocal via a single approximate DVE op;
    normalize + hi/lo split of attn for the o_proj feeds gpsimd/DVE.
  * o_proj emitted into a deferred queue drained between attention
    blocks so its matmuls fill tensor-queue gaps; fp16 output partials
    (host sums in f32).

Device layout (host-pretiled, partition-major; j indexes fp8 planes):
  xt  [128,16,2,2048] e4 : xt[p,ko,j,s] = (hi,lo)[j] of hidden[b,s,ko*128+p]
  wq  [128,16,2, 512] e4 : wq[p,ko,j,o] = (lo,hi)[j] of 256*w_qkv[q_rows[o],ko*128+p]
  wk,wv same as wq (k_rows / v_rows)
  wo  [128, 4,2,2048] e4 : wo[p,kb,j,o] = (lo,hi)[j] of 256*w_o[o,cols[kb*128+p]]
  outt[128,16,2048] f16 : outt[p,ot,s] = outT_partial[ot*128+p, s]

Toolchain quirks worked around here (walrus 1-sync-wait slots):
  - chunked tail drain monkeypatch; NoOp splitting of multi-waits
  - f32r matmul operands must be produced as f32r by ACT/DVE
  - lower_extended_insts() for custom-DVE ISA instruction bytes
"""
import numpy as np

import concourse.bass as bass
import concourse.mybir as mybir
import concourse.tile as tile
from concourse.bass_utils import run_bass_kernel_spmd
from concourse.vector_clock import ScopedClock, VectorClock

P = 128
S = 2048
H = 2048
NH_LOCAL = 4          # heads per core
KO = H // P           # 16 contraction chunks for the projections
SQ = 512              # q chunk width
NQC = S // SQ         # 4 q chunks
NKB = S // P          # 16 key blocks
F32 = mybir.dt.float32
F32R = mybir.dt.float32r
F16 = mybir.dt.float16
BF = mybir.dt.bfloat16
F8 = mybir.dt.float8e4
DR = mybir.MatmulPerfMode.DoubleRow
AF = mybir.ActivationFunctionType
SCALE = 1.0 / float(np.sqrt(128.0))

XCH = 512             # x chunk width in phase 1
NXCH = S // XCH       # 4 chunks
HI_X, LO_X = 0, 1     # x / attn plane order
LO_W, HI_W = 0, 1     # weight plane order


def _drain_and_barrier_chunked(self, tick_clock, wait_clock, _MAX=1):
    """Split the kernel-tail drain's waits: walrus allows only one sync
    wait per instruction in this toolchain."""
    g = tick_clock.global_clock
    n = len(g)
    vals = [g[i] for i in range(n)]
    nz = [i for i, v in enumerate(vals) if v > 0]
    chunks = [nz[i:i + _MAX] for i in range(0, len(nz), _MAX)] or [[]]
    for chunk in chunks:
        vec = [vals[i] if i in chunk else 0 for i in range(n)]
        d = self.nc.sync.drain()
        wait_clock.add_sem_waits(d.ins, ScopedClock({None: VectorClock(vec)}))
    self.nc.all_engine_barrier()
    assert self.sems is not None
    popped = self.nc._tile_sem_poison_stack.pop()
    assert popped is self._sem_poison
    self.nc.clear_and_free_semaphores(list(self.sems.allocated().values()))
    self.nc.all_engine_barrier()


tile.TileContext._drain_and_barrier = _drain_and_barrier_chunked


def _split_multi_waits(nc):
    """walrus allows ONE sync wait per instruction: hoist extra waits onto
    same-engine NoOps inserted directly before the offending instruction
    (identical semantics — the engine queue blocks on each in turn)."""
    ctr = 0
    for f in nc.m.functions:
        for blk in f.blocks:
            new = []
            changed = False
            for inst in blk.instructions:
                si = inst.sync_info
                waits = list(si.on_wait) if si and si.on_wait else []
                if len(waits) > 1:
                    changed = True
                    for w in waits[:-1]:
                        ctr += 1
                        nop = mybir.InstNoOp(name=f"I-wsplit-{ctr}",
                                             engine=inst.engine,
                                             ins=[], outs=[])
                        nop.sync_info = mybir.SyncInfo(on_wait=[w],
                                                       on_update=[])
                        new.append(nop)
                    ups = list(si.on_update) if si.on_update else []
                    inst.sync_info = mybir.SyncInfo(on_wait=[waits[-1]],
                                                   on_update=ups)
                new.append(inst)
            if changed:
                blk.instructions = new
    return ctr


def build():
    nc = bass.Bass()
    xt = nc.dram_tensor("xt", [P, KO, S], F16, kind="ExternalInput")
    wq = nc.dram_tensor("wq", [P, KO, NH_LOCAL * P], F16, kind="ExternalInput")
    wk = nc.dram_tensor("wk", [P, KO, NH_LOCAL * P], F16, kind="ExternalInput")
    wv = nc.dram_tensor("wv", [P, KO, NH_LOCAL * P], F16, kind="ExternalInput")
    wo = nc.dram_tensor("wo", [P, NH_LOCAL, S], F16, kind="ExternalInput")
    outt = nc.dram_tensor("outt", [P, KO, S], F16, kind="ExternalOutput")

    with tile.TileContext(nc) as tc:
        from contextlib import ExitStack
        with ExitStack() as ctx:
            const = ctx.enter_context(tc.tile_pool(name="const", bufs=1))

            # ---- constants -------------------------------------------------
            ones_f = const.tile([P, 1], F32)
            nc.vector.memset(ones_f[:], 1.0)
            ones_b = const.tile([P, 1], BF)
            nc.scalar.copy(ones_b[:], ones_f[:])
            onesrow_f = const.tile([1, P], F32)
            nc.vector.memset(onesrow_f[:], 1.0)
            onesrow_r = const.tile([1, P], F32R)
            nc.scalar.copy(onesrow_r[:], onesrow_f[:])
            obs_dve = const.tile([1, 1], F32)
            nc.vector.memset(obs_dve[:], 0.0)

            def _one(ap):
                return ap[tuple(slice(0, 1) for _ in ap.shape)]

            def dve_war_touch(ap):
                nc.vector.tensor_copy(_one(ap[:]), obs_dve[:])

            # ---- residents (live through phase 1+2) -----------------------
            qkv_pool = ctx.enter_context(tc.tile_pool(name="qkvp", bufs=1))
            # Q,K as qkvT: [d_in, o_tile(0-3 Q heads, 4-7 K heads), s],
            # fp16, scaled by SW (absorbed into the exp scale)
            qk_sb = qkv_pool.tile([P, 2 * NH_LOCAL, S], F16)
            # V as [s_in, s_tile, d_local], fp16, natural scale
            v_sb = qkv_pool.tile([P, NKB, NH_LOCAL * P], F16)

            # ================= phase 1: QKV projection =====================
            # fp8 DoubleRow with hi/lo planes: per k-block pair (k,k+1)
            # three DR instructions cover hi*hi(k)+hi*hi(k+1),
            # lo*hi(k)+hi*lo(k), lo*hi(k+1)+hi*lo(k+1).
            # w free layout: [0:512]=Q, [512:1024]=K, [1024:1536]=V
            with tc.tile_pool(name="p1w", bufs=1) as p1w, \
                 tc.tile_pool(name="p1x", bufs=3) as p1x, \
                 tc.tile_pool(name="p1ps", bufs=4, space="PSUM") as p1ps:

                w_r = p1w.tile([P, KO, 3 * NH_LOCAL * P], F16, tag="wr")
                x_tiles = []
                x_r0 = p1x.tile([P, KO, XCH], F16, tag="xr", name="xr0")
                for kq in range(4):
                    ks = slice(4 * kq, 4 * (kq + 1))
                    nc.sync.dma_start(x_r0[:, ks], xt.ap()[:, ks, 0:XCH])
                    nc.sync.dma_start(w_r[:, ks, 0:4 * P], wq.ap()[:, ks])
                x_tiles.append(x_r0)
                for i, wdram in ((1, wk), (2, wv)):
                    nc.sync.dma_start(
                        w_r[:, :, 4 * i * P:4 * (i + 1) * P], wdram.ap())

                for xc in range(NXCH):
                    if xc > 0:
                        x_r = p1x.tile([P, KO, XCH], F16, tag="xr")
                        nc.sync.dma_start(
                            x_r[:], xt.ap()[:, :, xc * XCH:(xc + 1) * XCH])
                    else:
                        x_r = x_tiles[0]

                    for ot in range(2 * NH_LOCAL):  # Q then K o-tiles
                        ps = p1ps.tile([P, XCH], F32, tag="p1qk")
                        for k in range(KO):
                            nc.tensor.matmul(
                                ps[:], w_r[:, k, ot * P:(ot + 1) * P],
                                x_r[:, k], start=(k == 0),
                                stop=(k == KO - 1))
                        nc.vector.tensor_copy(
                            qk_sb[:, ot, xc * XCH:(xc + 1) * XCH], ps[:])
                    # V: out [s_tile(128), d(512)] — copies on ACT
                    for st in range(XCH // P):
                        stg = xc * (XCH // P) + st
                        ps = p1ps.tile([P, NH_LOCAL * P], F32, tag="p1v")
                        for k in range(KO):
                            nc.tensor.matmul(
                                ps[:], x_r[:, k, st * P:(st + 1) * P],
                                w_r[:, k, 2 * NH_LOCAL * P:3 * NH_LOCAL * P],
                                start=(k == 0), stop=(k == KO - 1))
                        nc.scalar.copy(v_sb[:, stg, :], ps[:])

            # ============ phase 2+3: attention + interleaved o_proj ========
            attn_pool = ctx.enter_context(tc.tile_pool(name="attnp", bufs=1))
            attnT = attn_pool.tile([P, NH_LOCAL, S], F16)

            p3w = ctx.enter_context(tc.tile_pool(name="p3w", bufs=1))
            wo_r = p3w.tile([P, NH_LOCAL, S], F16)
            nc.sync.dma_start(wo_r[:], wo.ap())

            p2sb = ctx.enter_context(tc.tile_pool(name="p2sb", bufs=2))
            p2est = ctx.enter_context(tc.tile_pool(name="p2est", bufs=6))
            p2st = ctx.enter_context(
                tc.tile_pool(name="p2st", bufs=2, space="PSUM"))
            p2at = ctx.enter_context(
                tc.tile_pool(name="p2at", bufs=2, space="PSUM"))
            p2sm = ctx.enter_context(
                tc.tile_pool(name="p2sm", bufs=2, space="PSUM"))
            p3ps = ctx.enter_context(
                tc.tile_pool(name="p3ps", bufs=2, space="PSUM"))
            p3sb = ctx.enter_context(tc.tile_pool(name="p3sb", bufs=4))

            # Deferred-work queue: normalization chains and o_proj groups
            # are spread across later kb slots so their tensor ops fill
            # pipeline gaps instead of blocking the in-order tensor queue.
            pending = []
            slot = [0]

            def drain_slot():
                slot[0] += 1
                if pending and (len(pending) > 8 or slot[0] % 2 == 0):
                    pending.pop(0)()

            def make_norm(at_ps, den_r, h, qs):
                def norm():
                    rep_ps = p3ps.tile([P, SQ], F32, tag="p3ps")
                    nc.tensor.matmul(rep_ps[:], onesrow_r[:], den_r[:],
                                     start=True, stop=True)
                    rep_sb = p2sb.tile([P, SQ], F32, tag="repsb")
                    nc.vector.reciprocal_approx_fast(rep_sb[:], rep_ps[:])
                    nc.vector.tensor_mul(attnT[:, h, qs:qs + SQ],
                                         at_ps[:], rep_sb[:])
                return norm

            def make_oproj(sc, ot):
                def oproj():
                    ps = p3ps.tile([P, SQ], F32, tag="p3ps")
                    for kb in range(NH_LOCAL):
                        nc.tensor.matmul(
                            ps[:], wo_r[:, kb, ot * P:(ot + 1) * P],
                            attnT[:, kb, sc * SQ:(sc + 1) * SQ],
                            start=(kb == 0), stop=(kb == NH_LOCAL - 1))
                    stage = p3sb.tile([P, SQ], F16, tag="p3stage")
                    dve_war_touch(stage)
                    nc.vector.tensor_copy(stage[:], ps[:])
                    nc.sync.dma_start(
                        outt.ap()[:, ot, sc * SQ:(sc + 1) * SQ],
                        stage[:])
                return oproj

            def att_main(h, qc):
                nkb = 4 * (qc + 1)
                qs = qc * SQ

                at_ps = p2at.tile([P, SQ], F32, tag="atps")
                sm_ps = p2sm.tile([1, SQ], F32, tag="smps")

                st_tiles = {}

                def emit_st(kb):
                    st_ps = p2st.tile([P, SQ], F32, tag="stps")
                    nc.tensor.matmul(
                        st_ps[:],
                        qk_sb[:, NH_LOCAL + h, kb * P:(kb + 1) * P],
                        qk_sb[:, h, qs:qs + SQ],
                        start=True, stop=True)
                    st_tiles[kb] = st_ps

                emit_st(0)
                emit_st(1)
                for kb in range(nkb):
                    drain_slot()
                    if kb + 2 < nkb:
                        emit_st(kb + 2)
                    st_ps = st_tiles.pop(kb)
                    est = p2est.tile([P, SQ], BF, tag="est")
                    nc.scalar.activation(est[:], st_ps[:], AF.Exp,
                                         scale=SCALE)
                    if kb * P + P - 1 > qs:  # crosses the causal diagonal
                        nc.gpsimd.affine_select(
                            est[:], est[:], [[1, SQ]],
                            mybir.AluOpType.is_ge, 0.0,
                            base=qs - kb * P,
                            channel_multiplier=-1)
                    nc.tensor.matmul(sm_ps[:], ones_b[:], est[:],
                                     start=(kb == 0),
                                     stop=(kb == nkb - 1))
                    nc.tensor.matmul(
                        at_ps[:],
                        v_sb[:, kb, h * P:(h + 1) * P],
                        est[:],
                        start=(kb == 0), stop=(kb == nkb - 1))

                # denominators to f32r right away (ACT queue, lands just
                # after this head's last exp); the rest of the normalize
                # chain is deferred into the next head's kb slots.
                den_r = p2sb.tile([1, SQ], F32R, tag="denr")
                nc.scalar.copy(den_r[:], sm_ps[:])
                pending.append(make_norm(at_ps, den_r, h, qs))

            for qc in range(NQC):
                for h in range(NH_LOCAL):
                    att_main(h, qc)
                pending.extend(make_oproj(qc, ot) for ot in range(KO))
            while pending:
                pending.pop(0)()

    from concourse.library_overlay import lower_extended_insts
    lower_extended_insts(nc)   # populate .instr bytes for custom ISA ops
    _split_multi_waits(nc)
    return nc


_NC_CACHE = None


def _get_nc():
    global _NC_CACHE
    if _NC_CACHE is None:
        _NC_CACHE = build()
    return _NC_CACHE


def _prep_inputs(hidden_states, w_qkv, w_o):
    """Host-side shard + pre-tile + fp16-cast for the 8 cores."""
    F16_NP = np.float16
    hidden_states = np.asarray(hidden_states, dtype=np.float32)
    w_qkv = np.asarray(w_qkv, dtype=np.float32)
    w_o = np.asarray(w_o, dtype=np.float32)
    B = hidden_states.shape[0]

    in_maps = []
    xt_by_b = {}
    for b in range(B):
        # xt[p, ko, s] = hidden[b, s, ko*128+p]
        xt_by_b[b] = np.ascontiguousarray(
            hidden_states[b].T.reshape(KO, P, S).transpose(1, 0, 2)
        ).astype(F16_NP)
    for c in range(8):
        b = c // 4
        hs = [4 * (c % 4) + j for j in range(NH_LOCAL)]
        q_rows = np.concatenate([np.arange(h * P, (h + 1) * P) for h in hs])
        k_rows = q_rows + H
        v_rows = q_rows + 2 * H

        def wtile(rows):
            # [p, ko, o] = w_qkv[rows[o], ko*128+p]
            w = w_qkv[rows, :]                      # [512, 2048]
            return np.ascontiguousarray(
                w.T.reshape(KO, P, len(rows)).transpose(1, 0, 2)
            ).astype(F16_NP)

        # wo[p, kb, o] = w_o[o, cols[kb*128+p]]
        wo_c = np.ascontiguousarray(
            w_o[:, q_rows].T.reshape(NH_LOCAL, P, S).transpose(1, 0, 2)
        ).astype(F16_NP)
        in_maps.append({
            "xt": xt_by_b[b],
            "wq": wtile(q_rows),
            "wk": wtile(k_rows),
            "wv": wtile(v_rows),
            "wo": wo_c,
        })
    return in_maps


def run(hidden_states, w_qkv, w_o, trace=False, trace_cores=None):
    in_maps = _prep_inputs(hidden_states, w_qkv, w_o)
    nc = _get_nc()
    kwargs = {}
    if trace:
        kwargs["trace_cores"] = (trace_cores if trace_cores is not None
                                 else list(range(8)))
    res = run_bass_kernel_spmd(nc, in_maps, core_ids=list(range(8)),
                               trace=trace, **kwargs)
    B, S_, H_ = np.asarray(hidden_states).shape
    out = np.zeros((B, S_, H_), dtype=np.float32)
    for c in range(8):
        b = c // 4
        outt = res.results[c]["outt"]               # [128, 16, 2048] fp16
        outT = outt.astype(np.float32).transpose(1, 0, 2).reshape(H_, S_)
        out[b] += outT.T
    return out, res


def kernel(hidden_states, w_qkv, w_o):
    out, _ = run(hidden_states, w_qkv, w_o, trace=False)
    return out


# revision 27
# speedup vs baseline: 1.3548x; 1.0739x over previous
"""Tensor-parallel causal attention kernel for 8 trn2 NeuronCores.

Problem: B=2, S=2048, H=2048, 16 heads, head_dim=128 fp32.
  qkv = hidden @ w_qkv.T ; causal attention ; out = attn @ w_o.T

Sharding (hardcoded): core c in 0..7 handles batch b=c//4 and heads
hs = [4*(c%4) .. 4*(c%4)+3].  Each core computes a partial o_proj
output (contraction over its 512 hidden dims); the host sums the 4
partials per batch and transposes.  No device collectives.

v2 design:
  * Projections (qkv + o_proj) run as fp8-e4m3 DoubleRow matmuls on
    hi/lo decomposed operands: value = e4m3(hi) + e4m3(lo) carries
    ~11 significant bits.  Computing hi*hi (paired across two k-blocks
    per instruction) plus the two cross terms costs 0.75 cycles/row —
    1.33x faster than bf16/fp16 — at near-fp16 accuracy (the dropped
    lo*lo term is ~1e-4 relative).
  * Attention core (QK^T, exp, PV, row-sums) in fp16/bf16.
  * Causal mask post-exp on gpsimd; softmax denominators broadcast via
    a rank-1 f32r matmul; reciprocal via a single approximate DVE op;
    normalize + hi/lo split of attn for the o_proj feeds gpsimd/DVE.
  * o_proj emitted into a deferred queue drained between attention
    blocks so its matmuls fill tensor-queue gaps; fp16 output partials
    (host sums in f32).

Device layout (host-pretiled, partition-major; j indexes fp8 planes):
  xt  [128,16,2,2048] e4 : xt[p,ko,j,s] = (hi,lo)[j] of hidden[b,s,ko*128+p]
  wq  [128,16,2, 512] e4 : wq[p,ko,j,o] = (lo,hi)[j] of 256*w_qkv[q_rows[o],ko*128+p]
  wk,wv same as wq (k_rows / v_rows)
  wo  [128, 4,2,2048] e4 : wo[p,kb,j,o] = (lo,hi)[j] of 256*w_o[o,cols[kb*128+p]]
  outt[128,16,2048] f16 : outt[p,ot,s] = outT_partial[ot*128+p, s]

Toolchain quirks worked around here (walrus 1-sync-wait slots):
  - chunked tail drain monkeypatch; NoOp splitting of multi-waits
  - f32r matmul operands must be produced as f32r by ACT/DVE
  - lower_extended_insts() for custom-DVE ISA instruction bytes
"""
import numpy as np

import concourse.bass as bass
import concourse.mybir as mybir
import concourse.tile as tile
from concourse.bass_utils import run_bass_kernel_spmd
from concourse.vector_clock import ScopedClock, VectorClock

P = 128
S = 2048
H = 2048
NH_LOCAL = 4          # heads per core
KO = H // P           # 16 contraction chunks for the projections
SQ = 512              # q chunk width
NQC = S // SQ         # 4 q chunks
NKB = S // P          # 16 key blocks
F32 = mybir.dt.float32
F32R = mybir.dt.float32r
F16 = mybir.dt.float16
BF = mybir.dt.bfloat16
F8 = mybir.dt.float8e4
DR = mybir.MatmulPerfMode.DoubleRow
AF = mybir.ActivationFunctionType
SCALE = 1.0 / float(np.sqrt(128.0))

XCH = 512             # x chunk width in phase 1
NXCH = S // XCH       # 4 chunks
HI_X, LO_X = 0, 1     # x / attn plane order
LO_W, HI_W = 0, 1     # weight plane order


def _drain_and_barrier_chunked(self, tick_clock, wait_clock, _MAX=1):
    """Split the kernel-tail drain's waits: walrus allows only one sync
    wait per instruction in this toolchain."""
    g = tick_clock.global_clock
    n = len(g)
    vals = [g[i] for i in range(n)]
    nz = [i for i, v in enumerate(vals) if v > 0]
    chunks = [nz[i:i + _MAX] for i in range(0, len(nz), _MAX)] or [[]]
    for chunk in chunks:
        vec = [vals[i] if i in chunk else 0 for i in range(n)]
        d = self.nc.sync.drain()
        wait_clock.add_sem_waits(d.ins, ScopedClock({None: VectorClock(vec)}))
    self.nc.all_engine_barrier()
    assert self.sems is not None
    popped = self.nc._tile_sem_poison_stack.pop()
    assert popped is self._sem_poison
    self.nc.clear_and_free_semaphores(list(self.sems.allocated().values()))
    self.nc.all_engine_barrier()


tile.TileContext._drain_and_barrier = _drain_and_barrier_chunked


def _split_multi_waits(nc):
    """walrus allows ONE sync wait per instruction: hoist extra waits onto
    same-engine NoOps inserted directly before the offending instruction
    (identical semantics — the engine queue blocks on each in turn)."""
    ctr = 0
    for f in nc.m.functions:
        for blk in f.blocks:
            new = []
            changed = False
            for inst in blk.instructions:
                si = inst.sync_info
                waits = list(si.on_wait) if si and si.on_wait else []
                if len(waits) > 1:
                    changed = True
                    for w in waits[:-1]:
                        ctr += 1
                        nop = mybir.InstNoOp(name=f"I-wsplit-{ctr}",
                                             engine=inst.engine,
                                             ins=[], outs=[])
                        nop.sync_info = mybir.SyncInfo(on_wait=[w],
                                                       on_update=[])
                        new.append(nop)
                    ups = list(si.on_update) if si.on_update else []
                    inst.sync_info = mybir.SyncInfo(on_wait=[waits[-1]],
                                                   on_update=ups)
                new.append(inst)
            if changed:
                blk.instructions = new
    return ctr


def build():
    nc = bass.Bass()
    xt = nc.dram_tensor("xt", [P, KO, S], F16, kind="ExternalInput")
    wq = nc.dram_tensor("wq", [P, KO, NH_LOCAL * P], F16, kind="ExternalInput")
    wk = nc.dram_tensor("wk", [P, KO, NH_LOCAL * P], F16, kind="ExternalInput")
    wv = nc.dram_tensor("wv", [P, KO, NH_LOCAL * P], F16, kind="ExternalInput")
    wo = nc.dram_tensor("wo", [P, NH_LOCAL, S], F16, kind="ExternalInput")
    outt = nc.dram_tensor("outt", [P, KO, S], F16, kind="ExternalOutput")

    with tile.TileContext(nc) as tc:
        from contextlib import ExitStack
        with ExitStack() as ctx:
            const = ctx.enter_context(tc.tile_pool(name="const", bufs=1))

            # ---- constants -------------------------------------------------
            ones_f = const.tile([P, 1], F32)
            nc.vector.memset(ones_f[:], 1.0)
            ones_b = const.tile([P, 1], BF)
            nc.scalar.copy(ones_b[:], ones_f[:])
            onesrow_f = const.tile([1, P], F32)
            nc.vector.memset(onesrow_f[:], 1.0)
            onesrow_r = const.tile([1, P], F32R)
            nc.scalar.copy(onesrow_r[:], onesrow_f[:])
            obs_dve = const.tile([1, 1], F32)
            nc.vector.memset(obs_dve[:], 0.0)

            def _one(ap):
                return ap[tuple(slice(0, 1) for _ in ap.shape)]

            def dve_war_touch(ap):
                nc.vector.tensor_copy(_one(ap[:]), obs_dve[:])

            # ---- residents (live through phase 1+2) -----------------------
            qkv_pool = ctx.enter_context(tc.tile_pool(name="qkvp", bufs=1))
            # Q,K as qkvT: [d_in, o_tile(0-3 Q heads, 4-7 K heads), s],
            # fp16, scaled by SW (absorbed into the exp scale)
            qk_sb = qkv_pool.tile([P, 2 * NH_LOCAL, S], F16)
            # V as [s_in, s_tile, d_local], fp16, natural scale
            v_sb = qkv_pool.tile([P, NKB, NH_LOCAL * P], F16)

            # ================= phase 1: QKV projection =====================
            # fp8 DoubleRow with hi/lo planes: per k-block pair (k,k+1)
            # three DR instructions cover hi*hi(k)+hi*hi(k+1),
            # lo*hi(k)+hi*lo(k), lo*hi(k+1)+hi*lo(k+1).
            # w free layout: [0:512]=Q, [512:1024]=K, [1024:1536]=V
            with tc.tile_pool(name="p1w", bufs=1) as p1w, \
                 tc.tile_pool(name="p1x", bufs=3) as p1x, \
                 tc.tile_pool(name="p1ps", bufs=4, space="PSUM") as p1ps:

                w_r = p1w.tile([P, KO, 3 * NH_LOCAL * P], F16, tag="wr")
                x_tiles = []
                x_r0 = p1x.tile([P, KO, XCH], F16, tag="xr", name="xr0")
                for kq in range(4):
                    ks = slice(4 * kq, 4 * (kq + 1))
                    nc.sync.dma_start(x_r0[:, ks], xt.ap()[:, ks, 0:XCH])
                    nc.sync.dma_start(w_r[:, ks, 0:4 * P], wq.ap()[:, ks])
                x_tiles.append(x_r0)
                for i, wdram in ((1, wk), (2, wv)):
                    nc.sync.dma_start(
                        w_r[:, :, 4 * i * P:4 * (i + 1) * P], wdram.ap())

                for xc in range(NXCH):
                    if xc > 0:
                        x_r = p1x.tile([P, KO, XCH], F16, tag="xr")
                        nc.sync.dma_start(
                            x_r[:], xt.ap()[:, :, xc * XCH:(xc + 1) * XCH])
                    else:
                        x_r = x_tiles[0]

                    for ot in range(2 * NH_LOCAL):  # Q then K o-tiles
                        ps = p1ps.tile([P, XCH], F32, tag="p1qk")
                        for k in range(KO):
                            nc.tensor.matmul(
                                ps[:], w_r[:, k, ot * P:(ot + 1) * P],
                                x_r[:, k], start=(k == 0),
                                stop=(k == KO - 1))
                        nc.vector.tensor_copy(
                            qk_sb[:, ot, xc * XCH:(xc + 1) * XCH], ps[:])
                    # V: out [s_tile(128), d(512)] — copies on ACT
                    for st in range(XCH // P):
                        stg = xc * (XCH // P) + st
                        ps = p1ps.tile([P, NH_LOCAL * P], F32, tag="p1v")
                        for k in range(KO):
                            nc.tensor.matmul(
                                ps[:], x_r[:, k, st * P:(st + 1) * P],
                                w_r[:, k, 2 * NH_LOCAL * P:3 * NH_LOCAL * P],
                                start=(k == 0), stop=(k == KO - 1))
                        nc.scalar.copy(v_sb[:, stg, :], ps[:])

            # ============ phase 2+3: attention + interleaved o_proj ========
            attn_pool = ctx.enter_context(tc.tile_pool(name="attnp", bufs=1))
            attnT = attn_pool.tile([P, NH_LOCAL, S], F16)

            p3w = ctx.enter_context(tc.tile_pool(name="p3w", bufs=1))
            wo_r = p3w.tile([P, NH_LOCAL, S], F16)
            nc.sync.dma_start(wo_r[:], wo.ap())

            p2sb = ctx.enter_context(tc.tile_pool(name="p2sb", bufs=2))
            p2est = ctx.enter_context(tc.tile_pool(name="p2est", bufs=6))
            p2st = ctx.enter_context(
                tc.tile_pool(name="p2st", bufs=3, space="PSUM"))
            p2at = ctx.enter_context(
                tc.tile_pool(name="p2at", bufs=2, space="PSUM"))
            p2sm = ctx.enter_context(
                tc.tile_pool(name="p2sm", bufs=1, space="PSUM"))
            p3ps = ctx.enter_context(
                tc.tile_pool(name="p3ps", bufs=2, space="PSUM"))
            p3sb = ctx.enter_context(tc.tile_pool(name="p3sb", bufs=4))

            # Deferred-work queue: normalization chains and o_proj groups
            # are spread across later kb slots so their tensor ops fill
            # pipeline gaps instead of blocking the in-order tensor queue.
            pending_norm = []
            pending_oproj = []
            slot = [0]

            def drain_slot():
                slot[0] += 1
                while pending_norm:
                    pending_norm.pop(0)()     # norms gate psum reuse: ASAP
                if pending_oproj and (len(pending_oproj) > 12
                                      or slot[0] % 2 == 0):
                    pending_oproj.pop(0)()

            def make_norm(at_ps, den_r, h, qs):
                def norm():
                    rep_ps = p3ps.tile([P, SQ], F32, tag="p3ps")
                    nc.tensor.matmul(rep_ps[:], onesrow_r[:], den_r[:],
                                     start=True, stop=True)
                    rep_sb = p2sb.tile([P, SQ], F32, tag="repsb")
                    nc.vector.reciprocal_approx_fast(rep_sb[:], rep_ps[:])
                    nc.vector.tensor_mul(attnT[:, h, qs:qs + SQ],
                                         at_ps[:], rep_sb[:])
                return norm

            def make_oproj(sc, ot):
                def oproj():
                    ps = p3ps.tile([P, SQ], F32, tag="p3ps")
                    for kb in range(NH_LOCAL):
                        nc.tensor.matmul(
                            ps[:], wo_r[:, kb, ot * P:(ot + 1) * P],
                            attnT[:, kb, sc * SQ:(sc + 1) * SQ],
                            start=(kb == 0), stop=(kb == NH_LOCAL - 1))
                    stage = p3sb.tile([P, SQ], F16, tag="p3stage")
                    dve_war_touch(stage)
                    nc.vector.tensor_copy(stage[:], ps[:])
                    nc.sync.dma_start(
                        outt.ap()[:, ot, sc * SQ:(sc + 1) * SQ],
                        stage[:])
                return oproj

            def att_main(h, qc):
                nkb = 4 * (qc + 1)
                qs = qc * SQ

                at_ps = p2at.tile([P, SQ], F32, tag="atps")
                sm_ps = p2sm.tile([1, SQ], F32, tag="smps")

                st_tiles = {}

                def emit_st(kb):
                    st_ps = p2st.tile([P, SQ], F32, tag="stps")
                    nc.tensor.matmul(
                        st_ps[:],
                        qk_sb[:, NH_LOCAL + h, kb * P:(kb + 1) * P],
                        qk_sb[:, h, qs:qs + SQ],
                        start=True, stop=True)
                    st_tiles[kb] = st_ps

                emit_st(0)
                emit_st(1)
                emit_st(2) if nkb > 2 else None
                for kb in range(nkb):
                    drain_slot()
                    if kb + 3 < nkb:
                        emit_st(kb + 3)
                    st_ps = st_tiles.pop(kb)
                    est = p2est.tile([P, SQ], BF, tag="est")
                    nc.scalar.activation(est[:], st_ps[:], AF.Exp,
                                         scale=SCALE)
                    if kb * P + P - 1 > qs:  # crosses the causal diagonal
                        nc.gpsimd.affine_select(
                            est[:], est[:], [[1, SQ]],
                            mybir.AluOpType.is_ge, 0.0,
                            base=qs - kb * P,
                            channel_multiplier=-1)
                    nc.tensor.matmul(sm_ps[:], ones_b[:], est[:],
                                     start=(kb == 0),
                                     stop=(kb == nkb - 1))
                    nc.tensor.matmul(
                        at_ps[:],
                        v_sb[:, kb, h * P:(h + 1) * P],
                        est[:],
                        start=(kb == 0), stop=(kb == nkb - 1))

                # denominators to f32r right away (ACT queue, lands just
                # after this head's last exp); the rest of the normalize
                # chain is deferred into the next head's kb slots.
                den_r = p2sb.tile([1, SQ], F32R, tag="denr")
                nc.scalar.copy(den_r[:], sm_ps[:])
                pending_norm.append(make_norm(at_ps, den_r, h, qs))

            for qc in range(NQC):
                for h in range(NH_LOCAL):
                    att_main(h, qc)
                pending_oproj.extend(make_oproj(qc, ot) for ot in range(KO))
            while pending_norm:
                pending_norm.pop(0)()
            while pending_oproj:
                pending_oproj.pop(0)()

    from concourse.library_overlay import lower_extended_insts
    lower_extended_insts(nc)   # populate .instr bytes for custom ISA ops
    _split_multi_waits(nc)
    return nc


_NC_CACHE = None


def _get_nc():
    global _NC_CACHE
    if _NC_CACHE is None:
        _NC_CACHE = build()
    return _NC_CACHE


def _prep_inputs(hidden_states, w_qkv, w_o):
    """Host-side shard + pre-tile + fp16-cast for the 8 cores."""
    F16_NP = np.float16
    hidden_states = np.asarray(hidden_states, dtype=np.float32)
    w_qkv = np.asarray(w_qkv, dtype=np.float32)
    w_o = np.asarray(w_o, dtype=np.float32)
    B = hidden_states.shape[0]

    in_maps = []
    xt_by_b = {}
    for b in range(B):
        # xt[p, ko, s] = hidden[b, s, ko*128+p]
        xt_by_b[b] = np.ascontiguousarray(
            hidden_states[b].T.reshape(KO, P, S).transpose(1, 0, 2)
        ).astype(F16_NP)
    for c in range(8):
        b = c // 4
        hs = [4 * (c % 4) + j for j in range(NH_LOCAL)]
        q_rows = np.concatenate([np.arange(h * P, (h + 1) * P) for h in hs])
        k_rows = q_rows + H
        v_rows = q_rows + 2 * H

        def wtile(rows):
            # [p, ko, o] = w_qkv[rows[o], ko*128+p]
            w = w_qkv[rows, :]                      # [512, 2048]
            return np.ascontiguousarray(
                w.T.reshape(KO, P, len(rows)).transpose(1, 0, 2)
            ).astype(F16_NP)

        # wo[p, kb, o] = w_o[o, cols[kb*128+p]]
        wo_c = np.ascontiguousarray(
            w_o[:, q_rows].T.reshape(NH_LOCAL, P, S).transpose(1, 0, 2)
        ).astype(F16_NP)
        in_maps.append({
            "xt": xt_by_b[b],
            "wq": wtile(q_rows),
            "wk": wtile(k_rows),
            "wv": wtile(v_rows),
            "wo": wo_c,
        })
    return in_maps


def run(hidden_states, w_qkv, w_o, trace=False, trace_cores=None):
    in_maps = _prep_inputs(hidden_states, w_qkv, w_o)
    nc = _get_nc()
    kwargs = {}
    if trace:
        kwargs["trace_cores"] = (trace_cores if trace_cores is not None
                                 else list(range(8)))
    res = run_bass_kernel_spmd(nc, in_maps, core_ids=list(range(8)),
                               trace=trace, **kwargs)
    B, S_, H_ = np.asarray(hidden_states).shape
    out = np.zeros((B, S_, H_), dtype=np.float32)
    for c in range(8):
        b = c // 4
        outt = res.results[c]["outt"]               # [128, 16, 2048] fp16
        outT = outt.astype(np.float32).transpose(1, 0, 2).reshape(H_, S_)
        out[b] += outT.T
    return out, res


def kernel(hidden_states, w_qkv, w_o):
    out, _ = run(hidden_states, w_qkv, w_o, trace=False)
    return out
